# revision 27
# baseline (speedup 1.0000x reference)
"""Causal self-attention (B=4, T=2048, D=1024, H=16) on 8 TRN2 NeuronCores.

Sharding: tensor-parallel over 4 head-groups x data-parallel over 2 batch-groups.
Core c handles batches [2*(c//4), 2*(c//4)+2) and heads [4*(c%4), 4*(c%4)+4).
Each core computes a partial output projection (its 256 feature rows of W_proj);
the host sums the 4 head-group partials per batch group.

All matmul inputs are bf16 (inputs RNE-rounded on the host); accumulation is
fp32 in PSUM. bf16 streams at 1 cycle/row at EVERY free size (fp32r drops to
4 cycles/row below N=256), transposes at 1.0 vs 1.5 cycles/row, DVE/ACT get
2x 16-bit throughput, and HBM traffic halves. Softmax skips max-subtraction
(scores ~N(0,1)) so softmax(s) = exp(s)/sum(exp(s)).

Layout tricks:
- Q^T and K^T are packed two heads per 128 partitions; S^T contracts K=64
  partitions (the live head) directly, so K needs no zero-padding.
- V blocks are [64 V cols | 64 ones cols]: the PV matmul then produces the
  softmax denominator replicated across PSUM rows 64:128 for free, so the
  per-head normalization is reciprocal + multiply straight out of PSUM (no
  broadcast matmul on the tensor queue).
- Output projection for q-chunk j is emitted one j late so its PSUM/DVE work
  overlaps attention of chunk j+1.
- PSUM->SBUF evacuations are spread over scalar/vector/gpsimd so no single
  engine rate-limits the PE; x chunks are DMA-prefetched (incl. across the
  batch boundary) and the first x chunk is issued before the weights.
"""
import functools
from contextlib import ExitStack

import numpy as np
import ml_dtypes

import concourse.bacc as bacc
import concourse.tile as tile
import concourse.mybir as mybir
from concourse.bass_utils import run_bass_kernel_spmd
from concourse.masks import make_upper_triangular

F32 = mybir.dt.float32
F32R = mybir.dt.float32r
BF16 = mybir.dt.bfloat16
EXP = mybir.ActivationFunctionType.Exp

B, T, D, H, HD = 4, 2048, 1024, 16, 64
NB, NH = 2, 4            # batches / heads per core
DL = NH * HD             # local feature dim (256)
NC = 8
WCOL = 768               # per-dk weight columns: Q(256) K(256) V(256) packed


@functools.lru_cache(maxsize=1)
def build():
    nc = bacc.Bacc("TRN2", target_bir_lowering=False, debug=False, num_devices=NC)
    x_d = nc.dram_tensor("x", [NB, T, D], BF16, kind="ExternalInput").ap()
    wqkv_d = nc.dram_tensor("wqkv", [D, WCOL], BF16, kind="ExternalInput").ap()
    wproj_d = nc.dram_tensor("wproj", [DL, D], BF16, kind="ExternalInput").ap()
    ident_d = nc.dram_tensor("ident", [128, 128], BF16, kind="ExternalInput").ap()
    out_d = nc.dram_tensor("out", [NB, T, D], F32, kind="ExternalOutput").ap()

    NT5 = T // 512           # 4  (512-token super chunks)
    NTT = T // 128           # 16 (128-token chunks)
    NDK = D // 128           # 8  (feature chunks of input dim)

    with tile.TileContext(nc) as tc, ExitStack() as ctx:
        const = ctx.enter_context(tc.tile_pool(name="const", bufs=1))
        wpool = ctx.enter_context(tc.tile_pool(name="w", bufs=1))
        xin_pool = ctx.enter_context(tc.tile_pool(name="xin", bufs=4))

        ident = const.tile([128, 128], BF16)
        nc.sync.dma_start(ident[:], ident_d)

        # x chunk prefetch: one [256 tokens, D] chunk per (b, t5, half)
        xa_map = {}

        def xa_fetch(b, t5, half, eng):
            key = (b, t5, half)
            if key not in xa_map:
                xa = xin_pool.tile([128, 2 * D], BF16, tag="xa",
                                   name=f"xa{b}_{t5}_{half}")
                eng.dma_start(
                    xa[:].rearrange("p (a c) -> p a c", a=2),
                    x_d[b, 512 * t5 + 256 * half:512 * t5 + 256 * half + 256]
                    .rearrange("(a p) c -> p a c", p=128))
                xa_map[key] = xa
            return xa_map[key]

        xa_fetch(0, 0, 0, nc.scalar)
        xa_fetch(0, 0, 1, nc.gpsimd)

        # weights: w_sb[:, dk*WCOL + c] = wqkv[dk*128 + p, c].
        # Q/K columns land first so the first Q^T matmuls can start while the
        # V columns are still in flight.
        w_sb = wpool.tile([128, NDK * WCOL], BF16)
        nc.sync.dma_start(
            w_sb[:].rearrange("p (a c) -> p a c", a=NDK)[:, :, 0:512],
            wqkv_d.rearrange("(a p) c -> p a c", p=128)[:, :, 0:512])
        nc.sync.dma_start(
            w_sb[:].rearrange("p (a c) -> p a c", a=NDK)[:, :, 512:768],
            wqkv_d.rearrange("(a p) c -> p a c", p=128)[:, :, 512:768])
        wp_sb = wpool.tile([128, 2 * D], BF16)
        nc.sync.dma_start(
            wp_sb[:].rearrange("p (a c) -> p a c", a=2),
            wproj_d.rearrange("(a p) c -> p a c", p=128))

        tri32 = const.tile([128, 128], F32)   # tri32[k,q] = 1.0 iff q >= k
        make_upper_triangular(nc, tri32[:], val=1.0, diag=True)
        tri = const.tile([128, 128], BF16)
        nc.vector.tensor_copy(tri[:], tri32[:])
        onesb = const.tile([128, 1024], BF16)
        nc.gpsimd.memset(onesb[:], 1.0)

        for b in range(NB):
            with tc.tile_pool(name="actv", bufs=1) as actv:
                # Q^T / K^T packed: 2 chunks of 128 rows (2 heads each)
                qt = [actv.tile([128, T], BF16, tag=f"qt{cc}", name=f"qt{cc}")
                      for cc in range(2)]
                kt = [actv.tile([128, T], BF16, tag=f"kt{cc}", name=f"kt{cc}")
                      for cc in range(2)]
                # V blocks per (token-tile ti, head h): 128 cols at (ti*4+h)*128:
                # cols 0-63 = V, cols 64-127 = ones (denominator replication)
                v_sb = actv.tile([128, NTT * NH * 128], BF16, tag="v")
                v128 = v_sb[:].rearrange("p (n c) -> p n c", c=128)

                # ---- Phase A: x^T (PE transpose), Q^T, K^T, V ----
                with tc.tile_pool(name="xt", bufs=2) as xt_pool, \
                     tc.tile_pool(name="psT", bufs=4, space="PSUM") as psT, \
                     tc.tile_pool(name="psQK", bufs=2, space="PSUM") as psQK, \
                     tc.tile_pool(name="psV", bufs=2, space="PSUM") as psV:

                    def emit_transposes(t5):
                        xas = [xa_fetch(b, t5, half, nc.scalar)
                               for half in range(2)]
                        nb_, nt5 = (b, t5 + 1) if t5 + 1 < NT5 else (b + 1, 0)
                        if nb_ < NB:
                            for half in range(2):
                                xa_fetch(nb_, nt5, half, nc.scalar)
                        xt = [xt_pool.tile([128, 512], BF16, tag=f"xt{dk}",
                                           name=f"xt{dk}") for dk in range(NDK)]
                        # x^T: 2 transposes into one PSUM tile, 1 evac each
                        ev = 0
                        for half in range(2):
                            for dk in range(NDK):
                                pt = psT.tile([128, 256], BF16, tag="pt")
                                for tt in range(2):
                                    nc.tensor.matmul(
                                        pt[:, tt * 128:tt * 128 + 128],
                                        xas[half][:, tt * D + dk * 128:tt * D + dk * 128 + 128],
                                        ident[:], is_transpose=True,
                                        start=(tt == 0), stop=(tt == 1))
                                if ev % 2 == 0:
                                    nc.scalar.copy(
                                        xt[dk][:, 256 * half:256 * half + 256], pt[:])
                                else:
                                    nc.vector.tensor_copy(
                                        xt[dk][:, 256 * half:256 * half + 256], pt[:])
                                ev += 1
                        return xt

                    def emit_qkv(t5, xt):
                        for cc in range(2):     # Q^T
                            ps = psQK.tile([128, 512], F32, tag="qk")
                            for dk in range(NDK):
                                nc.tensor.matmul(
                                    ps[:],
                                    w_sb[:, dk * WCOL + cc * 128:dk * WCOL + cc * 128 + 128],
                                    xt[dk][:],
                                    start=(dk == 0), stop=(dk == NDK - 1))
                            nc.vector.tensor_copy(
                                qt[cc][:, t5 * 512:t5 * 512 + 512], ps[:])
                        for cc in range(2):     # K^T packed (same 2-head layout)
                            ps = psQK.tile([128, 512], F32, tag="qk")
                            for dk in range(NDK):
                                nc.tensor.matmul(
                                    ps[:],
                                    w_sb[:, dk * WCOL + 256 + cc * 128:dk * WCOL + 256 + cc * 128 + 128],
                                    xt[dk][:],
                                    start=(dk == 0), stop=(dk == NDK - 1))
                            nc.vector.tensor_copy(
                                kt[cc][:, t5 * 512:t5 * 512 + 512], ps[:])
                        for tt in range(4):     # V packed (N=256)
                            ps = psV.tile([128, 256], F32, tag="v")
                            for dk in range(NDK):
                                nc.tensor.matmul(
                                    ps[:],
                                    xt[dk][:, tt * 128:tt * 128 + 128],
                                    w_sb[:, dk * WCOL + 512:dk * WCOL + 768],
                                    start=(dk == 0), stop=(dk == NDK - 1))
                            ti = t5 * 4 + tt
                            nc.vector.tensor_copy(
                                v128[:, ti * 4:ti * 4 + 4, 64:128],
                                ps[:].rearrange("p (n c) -> p n c", c=64))
                        # ones cols 0:64 of each 128-block of this chunk (the
                        # denominator must land at PSUM partition offset 0:
                        # custom-DVE reciprocal drops nonzero partition offsets)
                        nc.vector.tensor_copy(
                            v128[:, 16 * t5:16 * (t5 + 1), 0:64],
                            onesb[:].rearrange("p (n c) -> p n c", c=64))

                    # transposes of t5+1 run ahead of QKV(t5): extra PE work
                    # covers the initial weight-DMA latency
                    xt_prev = emit_transposes(0)
                    for t5 in range(NT5):
                        xt_next = (emit_transposes(t5 + 1)
                                   if t5 + 1 < NT5 else None)
                        emit_qkv(t5, xt_prev)
                        xt_prev = xt_next

                # ---- Phase B: attention + (pipelined) output projection ----
                # All S^T matmuls of a (h,j) first (exps stream on ACT), then
                # all PV matmuls: long same-shape runs keep the PE at full clock.
                with tc.tile_pool(name="psS", bufs=2, space="PSUM") as psS_pool, \
                     tc.tile_pool(name="psY", bufs=2, space="PSUM") as psY_pool, \
                     tc.tile_pool(name="psO", bufs=2, space="PSUM") as psO_pool, \
                     tc.tile_pool(name="pP", bufs=18) as pP, \
                     tc.tile_pool(name="ytp", bufs=2) as ytp, \
                     tc.tile_pool(name="ost", bufs=2) as ost_pool, \
                     tc.tile_pool(name="rbp", bufs=3) as rbp:

                    def emit_proj(j, yt):
                        ev = 0
                        for g2 in range(2):
                            ostage = ost_pool.tile([128, 2 * D], F32, tag="o")
                            for a in range(2):
                                tt = 2 * g2 + a
                                for nn2 in range(2):
                                    ps = psO_pool.tile([128, 512], F32, tag="o")
                                    for ff in range(2):
                                        nc.tensor.matmul(
                                            ps[:],
                                            yt[ff][:, 128 * tt:128 * tt + 128],
                                            wp_sb[:, ff * D + 512 * nn2:ff * D + 512 * nn2 + 512],
                                            start=(ff == 0), stop=(ff == 1))
                                    dst = ostage[:, a * D + 512 * nn2:a * D + 512 * nn2 + 512]
                                    if ev % 2 == 0:
                                        nc.vector.tensor_copy(dst, ps[:])
                                    else:
                                        nc.scalar.copy(dst, ps[:])
                                    ev += 1
                            nc.sync.dma_start(
                                out_d[b, 512 * j + 256 * g2:512 * j + 256 * g2 + 256]
                                .rearrange("(a p) c -> p a c", p=128),
                                ostage[:].rearrange("p (a c) -> p a c", a=2))

                    # prefetch next batch's first x chunk during attention
                    if b + 1 < NB:
                        for half in range(2):
                            xa_fetch(b + 1, 0, half, nc.gpsimd)

                    # pending = PV work of the previous head, interleaved into
                    # the next head's S chain so exp-free PV matmuls cover the
                    # ACT exp lag (the S chain is exp-throughput-bound).
                    pending = None

                    def pv_step(n):
                        # emit up to n PV matmuls of the pending head
                        if pending is None:
                            return
                        for _ in range(n):
                            i = pending["i"]
                            if i >= pending["nk"]:
                                return
                            pending["i"] = i + 1
                            off = pending["offs"][i]
                            nc.tensor.matmul(
                                pending["psY"][:, off:512],
                                v_sb[:, 512 * i + 128 * pending["h"]:
                                     512 * i + 128 * pending["h"] + 128],
                                pending["Ps"][i // 2][:, (i % 2) * 512 + off:
                                                      (i % 2 + 1) * 512],
                                start=(i == 0), stop=(i == pending["nk"] - 1))

                    def pv_finish():
                        nonlocal pending
                        if pending is None:
                            return
                        pv_step(pending["nk"])
                        psY = pending["psY"]
                        # normalize: denominator sits replicated in rows 0:64
                        # of psY (ones cols of V) — no broadcast matmul.
                        rb = rbp.tile([64, 512], F32, tag="rb")
                        nc.vector.reciprocal_approx_fast(rb[:], psY[0:64, :])
                        nc.vector.tensor_mul(
                            pending["yt"], psY[64:128, :], rb[:])
                        pending = None

                    prev = None
                    for j in range(NT5):
                        yt = [ytp.tile([128, 512], BF16, tag=f"yt{ff}",
                                       name=f"yt{ff}") for ff in range(2)]
                        for h in range(NH):
                            ro = 64 * (h % 2)
                            cc = h // 2
                            nk = 4 * j + 4
                            offs = [128 * (i - 4 * j) if i - 4 * j > 0 else 0
                                    for i in range(nk)]
                            npair = nk // 2
                            quota = ((pending["nk"] + npair - 1) // npair
                                     if pending is not None else 0)
                            Ps = []
                            for m in range(npair):
                                psS = psS_pool.tile([128, 1024], F32, tag="s",
                                                    name=f"psS{m}")
                                P = pP.tile([128, 1024], BF16, tag="p",
                                            name=f"P{m}")
                                Ps.append(P)
                                for c in (0, 1):
                                    i = 2 * m + c
                                    off = offs[i]
                                    nc.tensor.matmul(
                                        psS[:, c * 512 + off:(c + 1) * 512],
                                        kt[cc][ro:ro + 64, 128 * i:128 * i + 128],
                                        qt[cc][ro:ro + 64, 512 * j + off:512 * (j + 1)],
                                        start=True, stop=True)
                                pv_step(quota)
                                if 2 * m + 1 < 4 * j or 2 * m == 4 * j:
                                    # t0/t1 diagonal pair: exp the whole tile in
                                    # one op; cols 512..640 are never read by PV
                                    nc.scalar.activation(P[:], psS[:], EXP, scale=0.125)
                                else:
                                    for c in (0, 1):
                                        off = offs[2 * m + c]
                                        nc.scalar.activation(
                                            P[:, c * 512 + off:(c + 1) * 512],
                                            psS[:, c * 512 + off:(c + 1) * 512],
                                            EXP, scale=0.125)
                                for c in (0, 1):
                                    i = 2 * m + c
                                    if i >= 4 * j:
                                        off = offs[i]
                                        nc.vector.tensor_mul(
                                            P[:, c * 512 + off:c * 512 + off + 128],
                                            P[:, c * 512 + off:c * 512 + off + 128],
                                            tri[:])
                            pv_finish()
                            psY = psY_pool.tile([128, 512], F32, tag="y")
                            pending = {"psY": psY, "Ps": Ps, "nk": nk,
                                       "offs": offs, "h": h, "i": 0,
                                       "yt": yt[h // 2][ro:ro + 64, :]}
                        if prev is not None:
                            emit_proj(prev[0], prev[1])
                        prev = (j, yt)
                    pv_finish()
                    emit_proj(prev[0], prev[1])

    nc.compile()
    return nc


def to_bf16(a: np.ndarray) -> np.ndarray:
    return np.ascontiguousarray(a).astype(ml_dtypes.bfloat16)


def make_in_maps(x, W_qkv, W_proj):
    ident = np.eye(128, dtype=np.float32)
    in_maps = []
    for c in range(NC):
        bg, hg = c // 4, c % 4
        wq = np.concatenate(
            [W_qkv[:, 256 * hg:256 * hg + 256],
             W_qkv[:, 1024 + 256 * hg:1024 + 256 * hg + 256],
             W_qkv[:, 2048 + 256 * hg:2048 + 256 * hg + 256]], axis=1)
        in_maps.append({
            "x": to_bf16(x[2 * bg:2 * bg + 2]),
            "wqkv": to_bf16(wq),
            "wproj": to_bf16(W_proj[256 * hg:256 * hg + 256, :]),
            "ident": to_bf16(ident),
        })
    return in_maps


def kernel(x, W_qkv, W_proj):
    x = np.asarray(x, dtype=np.float32)
    W_qkv = np.asarray(W_qkv, dtype=np.float32)
    W_proj = np.asarray(W_proj, dtype=np.float32)
    nc = build()
    res = run_bass_kernel_spmd(nc, make_in_maps(x, W_qkv, W_proj), list(range(NC)))
    out = np.zeros((B, T, D), dtype=np.float64)
    for c in range(NC):
        bg = c // 4
        out[2 * bg:2 * bg + 2] += res.results[c]["out"].astype(np.float64)
    return out.astype(np.float32)


# revision 28
# speedup vs baseline: 1.1582x; 1.1582x over previous
"""Causal self-attention (B=4, T=2048, D=1024, H=16) on 8 TRN2 NeuronCores.

Sharding: tensor-parallel over 4 head-groups x data-parallel over 2 batch-groups.
Core c handles batches [2*(c//4), 2*(c//4)+2) and heads [4*(c%4), 4*(c%4)+4).
Each core computes a partial output projection (its 256 feature rows of W_proj);
the host sums the 4 head-group partials per batch group.

All matmul inputs are bf16 (inputs RNE-rounded on the host); accumulation is
fp32 in PSUM. bf16 streams at 1 cycle/row at EVERY free size (fp32r drops to
4 cycles/row below N=256), transposes at 1.0 vs 1.5 cycles/row, DVE/ACT get
2x 16-bit throughput, and HBM traffic halves. Softmax skips max-subtraction
(scores ~N(0,1)) so softmax(s) = exp(s)/sum(exp(s)).

Layout tricks:
- Q^T and K^T are packed two heads per 128 partitions; S^T contracts K=64
  partitions (the live head) directly, so K needs no zero-padding.
- V blocks are [64 V cols | 64 ones cols]: the PV matmul then produces the
  softmax denominator replicated across PSUM rows 64:128 for free, so the
  per-head normalization is reciprocal + multiply straight out of PSUM (no
  broadcast matmul on the tensor queue).
- Output projection for q-chunk j is emitted one j late so its PSUM/DVE work
  overlaps attention of chunk j+1.
- PSUM->SBUF evacuations are spread over scalar/vector/gpsimd so no single
  engine rate-limits the PE; x chunks are DMA-prefetched (incl. across the
  batch boundary) and the first x chunk is issued before the weights.
"""
import functools
from contextlib import ExitStack

import numpy as np
import ml_dtypes

import concourse.bacc as bacc
import concourse.tile as tile
import concourse.mybir as mybir
from concourse.bass_utils import run_bass_kernel_spmd
from concourse.masks import make_upper_triangular

F32 = mybir.dt.float32
F32R = mybir.dt.float32r
BF16 = mybir.dt.bfloat16
EXP = mybir.ActivationFunctionType.Exp

B, T, D, H, HD = 4, 2048, 1024, 16, 64
NB, NH = 2, 4            # batches / heads per core
DL = NH * HD             # local feature dim (256)
NC = 8
WCOL = 768               # per-dk weight columns: Q(256) K(256) V(256) packed


@functools.lru_cache(maxsize=1)
def build():
    nc = bacc.Bacc("TRN2", target_bir_lowering=False, debug=False, num_devices=NC)
    x_d = nc.dram_tensor("x", [NB, T, D], BF16, kind="ExternalInput").ap()
    wqkv_d = nc.dram_tensor("wqkv", [D, WCOL], BF16, kind="ExternalInput").ap()
    wproj_d = nc.dram_tensor("wproj", [DL, D], BF16, kind="ExternalInput").ap()
    ident_d = nc.dram_tensor("ident", [128, 128], BF16, kind="ExternalInput").ap()
    out_d = nc.dram_tensor("out", [NB, T, D], F32, kind="ExternalOutput").ap()

    NT5 = T // 512           # 4  (512-token super chunks)
    NTT = T // 128           # 16 (128-token chunks)
    NDK = D // 128           # 8  (feature chunks of input dim)

    with tile.TileContext(nc) as tc, ExitStack() as ctx:
        const = ctx.enter_context(tc.tile_pool(name="const", bufs=1))
        wpool = ctx.enter_context(tc.tile_pool(name="w", bufs=1))
        xin_pool = ctx.enter_context(tc.tile_pool(name="xin", bufs=4))

        ident = const.tile([128, 128], BF16)
        nc.sync.dma_start(ident[:], ident_d)

        # x chunk prefetch: one [256 tokens, D] chunk per (b, t5, half)
        xa_map = {}

        def xa_fetch(b, t5, half, eng):
            key = (b, t5, half)
            if key not in xa_map:
                xa = xin_pool.tile([128, 2 * D], BF16, tag="xa",
                                   name=f"xa{b}_{t5}_{half}")
                eng.dma_start(
                    xa[:].rearrange("p (a c) -> p a c", a=2),
                    x_d[b, 512 * t5 + 256 * half:512 * t5 + 256 * half + 256]
                    .rearrange("(a p) c -> p a c", p=128))
                xa_map[key] = xa
            return xa_map[key]

        xa_fetch(0, 0, 0, nc.scalar)
        xa_fetch(0, 0, 1, nc.scalar)

        # weights: w_sb[:, dk*WCOL + c] = wqkv[dk*128 + p, c]
        w_sb = wpool.tile([128, NDK * WCOL], BF16)
        nc.sync.dma_start(
            w_sb[:].rearrange("p (a c) -> p a c", a=NDK),
            wqkv_d.rearrange("(a p) c -> p a c", p=128))
        wp_sb = wpool.tile([128, 2 * D], BF16)
        nc.sync.dma_start(
            wp_sb[:].rearrange("p (a c) -> p a c", a=2),
            wproj_d.rearrange("(a p) c -> p a c", p=128))

        tri32 = const.tile([128, 128], F32)   # tri32[k,q] = 1.0 iff q >= k
        make_upper_triangular(nc, tri32[:], val=1.0, diag=True)
        tri = const.tile([128, 128], BF16)
        nc.vector.tensor_copy(tri[:], tri32[:])
        onesb = const.tile([128, 1024], BF16)
        nc.gpsimd.memset(onesb[:], 1.0)

        for b in range(NB):
            with tc.tile_pool(name="actv", bufs=1) as actv:
                # Q^T / K^T packed: 2 chunks of 128 rows (2 heads each)
                qt = [actv.tile([128, T], BF16, tag=f"qt{cc}", name=f"qt{cc}")
                      for cc in range(2)]
                kt = [actv.tile([128, T], BF16, tag=f"kt{cc}", name=f"kt{cc}")
                      for cc in range(2)]
                # V blocks per (token-tile ti, head h): 128 cols at (ti*4+h)*128:
                # cols 0-63 = V, cols 64-127 = ones (denominator replication)
                v_sb = actv.tile([128, NTT * NH * 128], BF16, tag="v")
                v128 = v_sb[:].rearrange("p (n c) -> p n c", c=128)

                # ---- Phase A: x^T (PE transpose), Q^T, K^T, V ----
                with tc.tile_pool(name="xt", bufs=2) as xt_pool, \
                     tc.tile_pool(name="psT", bufs=4, space="PSUM") as psT, \
                     tc.tile_pool(name="psQK", bufs=2, space="PSUM") as psQK, \
                     tc.tile_pool(name="psV", bufs=2, space="PSUM") as psV:

                    def emit_transposes(t5):
                        xas = [xa_fetch(b, t5, half, nc.scalar)
                               for half in range(2)]
                        nb_, nt5 = (b, t5 + 1) if t5 + 1 < NT5 else (b + 1, 0)
                        if nb_ < NB:
                            for half in range(2):
                                xa_fetch(nb_, nt5, half, nc.scalar)
                        xt = [xt_pool.tile([128, 512], BF16, tag=f"xt{dk}",
                                           name=f"xt{dk}") for dk in range(NDK)]
                        # x^T: 2 transposes into one PSUM tile, 1 evac each
                        ev = 0
                        for half in range(2):
                            for dk in range(NDK):
                                pt = psT.tile([128, 256], BF16, tag="pt")
                                for tt in range(2):
                                    nc.tensor.matmul(
                                        pt[:, tt * 128:tt * 128 + 128],
                                        xas[half][:, tt * D + dk * 128:tt * D + dk * 128 + 128],
                                        ident[:], is_transpose=True,
                                        start=(tt == 0), stop=(tt == 1))
                                if ev % 2 == 0:
                                    nc.scalar.copy(
                                        xt[dk][:, 256 * half:256 * half + 256], pt[:])
                                else:
                                    nc.vector.tensor_copy(
                                        xt[dk][:, 256 * half:256 * half + 256], pt[:])
                                ev += 1
                        return xt

                    def emit_qkv(t5, xt):
                        for cc in range(2):     # Q^T
                            ps = psQK.tile([128, 512], F32, tag="qk")
                            for dk in range(NDK):
                                nc.tensor.matmul(
                                    ps[:],
                                    w_sb[:, dk * WCOL + cc * 128:dk * WCOL + cc * 128 + 128],
                                    xt[dk][:],
                                    start=(dk == 0), stop=(dk == NDK - 1))
                            nc.vector.tensor_copy(
                                qt[cc][:, t5 * 512:t5 * 512 + 512], ps[:])
                        for cc in range(2):     # K^T packed (same 2-head layout)
                            ps = psQK.tile([128, 512], F32, tag="qk")
                            for dk in range(NDK):
                                nc.tensor.matmul(
                                    ps[:],
                                    w_sb[:, dk * WCOL + 256 + cc * 128:dk * WCOL + 256 + cc * 128 + 128],
                                    xt[dk][:],
                                    start=(dk == 0), stop=(dk == NDK - 1))
                            nc.vector.tensor_copy(
                                kt[cc][:, t5 * 512:t5 * 512 + 512], ps[:])
                        for tt in range(4):     # V packed (N=256)
                            ps = psV.tile([128, 256], F32, tag="v")
                            for dk in range(NDK):
                                nc.tensor.matmul(
                                    ps[:],
                                    xt[dk][:, tt * 128:tt * 128 + 128],
                                    w_sb[:, dk * WCOL + 512:dk * WCOL + 768],
                                    start=(dk == 0), stop=(dk == NDK - 1))
                            ti = t5 * 4 + tt
                            nc.vector.tensor_copy(
                                v128[:, ti * 4:ti * 4 + 4, 64:128],
                                ps[:].rearrange("p (n c) -> p n c", c=64))
                        # ones cols 0:64 of each 128-block of this chunk (the
                        # denominator must land at PSUM partition offset 0:
                        # custom-DVE reciprocal drops nonzero partition offsets)
                        nc.vector.tensor_copy(
                            v128[:, 16 * t5:16 * (t5 + 1), 0:64],
                            onesb[:].rearrange("p (n c) -> p n c", c=64))

                    for t5 in range(NT5):
                        emit_qkv(t5, emit_transposes(t5))

                # ---- Phase B: attention + (pipelined) output projection ----
                # All S^T matmuls of a (h,j) first (exps stream on ACT), then
                # all PV matmuls: long same-shape runs keep the PE at full clock.
                with tc.tile_pool(name="psS", bufs=2, space="PSUM") as psS_pool, \
                     tc.tile_pool(name="psY", bufs=2, space="PSUM") as psY_pool, \
                     tc.tile_pool(name="psO", bufs=2, space="PSUM") as psO_pool, \
                     tc.tile_pool(name="pP", bufs=9) as pP, \
                     tc.tile_pool(name="ytp", bufs=2) as ytp, \
                     tc.tile_pool(name="ost", bufs=2) as ost_pool, \
                     tc.tile_pool(name="rbp", bufs=3) as rbp:

                    def emit_proj(j, yt):
                        ev = 0
                        for g2 in range(2):
                            ostage = ost_pool.tile([128, 2 * D], F32, tag="o")
                            for a in range(2):
                                tt = 2 * g2 + a
                                for nn2 in range(2):
                                    ps = psO_pool.tile([128, 512], F32, tag="o")
                                    for ff in range(2):
                                        nc.tensor.matmul(
                                            ps[:],
                                            yt[ff][:, 128 * tt:128 * tt + 128],
                                            wp_sb[:, ff * D + 512 * nn2:ff * D + 512 * nn2 + 512],
                                            start=(ff == 0), stop=(ff == 1))
                                    dst = ostage[:, a * D + 512 * nn2:a * D + 512 * nn2 + 512]
                                    if ev % 2 == 0:
                                        nc.vector.tensor_copy(dst, ps[:])
                                    else:
                                        nc.scalar.copy(dst, ps[:])
                                    ev += 1
                            nc.sync.dma_start(
                                out_d[b, 512 * j + 256 * g2:512 * j + 256 * g2 + 256]
                                .rearrange("(a p) c -> p a c", p=128),
                                ostage[:].rearrange("p (a c) -> p a c", a=2))

                    # prefetch next batch's first x chunk during attention
                    if b + 1 < NB:
                        for half in range(2):
                            xa_fetch(b + 1, 0, half, nc.gpsimd)

                    prev = None
                    for j in range(NT5):
                        yt = [ytp.tile([128, 512], BF16, tag=f"yt{ff}",
                                       name=f"yt{ff}") for ff in range(2)]
                        for h in range(NH):
                            ro = 64 * (h % 2)
                            cc = h // 2
                            nk = 4 * j + 4
                            offs = [128 * (i - 4 * j) if i - 4 * j > 0 else 0
                                    for i in range(nk)]
                            Ps = []
                            for m in range(nk // 2):
                                psS = psS_pool.tile([128, 1024], F32, tag="s",
                                                    name=f"psS{m}")
                                P = pP.tile([128, 1024], BF16, tag="p",
                                            name=f"P{m}")
                                Ps.append(P)
                                for c in (0, 1):
                                    i = 2 * m + c
                                    off = offs[i]
                                    nc.tensor.matmul(
                                        psS[:, c * 512 + off:(c + 1) * 512],
                                        kt[cc][ro:ro + 64, 128 * i:128 * i + 128],
                                        qt[cc][ro:ro + 64, 512 * j + off:512 * (j + 1)],
                                        start=True, stop=True)
                                if 2 * m + 1 < 4 * j or 2 * m == 4 * j:
                                    # t0/t1 diagonal pair: exp the whole tile in
                                    # one op; cols 512..640 are never read by PV
                                    nc.scalar.activation(P[:], psS[:], EXP, scale=0.125)
                                else:
                                    for c in (0, 1):
                                        off = offs[2 * m + c]
                                        nc.scalar.activation(
                                            P[:, c * 512 + off:(c + 1) * 512],
                                            psS[:, c * 512 + off:(c + 1) * 512],
                                            EXP, scale=0.125)
                                for c in (0, 1):
                                    i = 2 * m + c
                                    if i >= 4 * j:
                                        off = offs[i]
                                        nc.vector.tensor_mul(
                                            P[:, c * 512 + off:c * 512 + off + 128],
                                            P[:, c * 512 + off:c * 512 + off + 128],
                                            tri[:])
                            psY = psY_pool.tile([128, 512], F32, tag="y")
                            for i in range(nk):
                                off = offs[i]
                                nc.tensor.matmul(
                                    psY[:, off:512],
                                    v_sb[:, 512 * i + 128 * h:512 * i + 128 * h + 128],
                                    Ps[i // 2][:, (i % 2) * 512 + off:(i % 2 + 1) * 512],
                                    start=(i == 0), stop=(i == nk - 1))
                            # normalize: denominator sits replicated in rows
                            # 0:64 of psY (ones cols of V) — no broadcast.
                            rb = rbp.tile([64, 512], F32, tag="rb")
                            nc.vector.reciprocal_approx_fast(rb[:], psY[0:64, :])
                            nc.vector.tensor_mul(
                                yt[h // 2][ro:ro + 64, :],
                                psY[64:128, :], rb[:])
                        if prev is not None:
                            emit_proj(prev[0], prev[1])
                        prev = (j, yt)
                    emit_proj(prev[0], prev[1])

    nc.compile()
    return nc


def to_bf16(a: np.ndarray) -> np.ndarray:
    return np.ascontiguousarray(a).astype(ml_dtypes.bfloat16)


def make_in_maps(x, W_qkv, W_proj):
    ident = np.eye(128, dtype=np.float32)
    in_maps = []
    for c in range(NC):
        bg, hg = c // 4, c % 4
        wq = np.concatenate(
            [W_qkv[:, 256 * hg:256 * hg + 256],
             W_qkv[:, 1024 + 256 * hg:1024 + 256 * hg + 256],
             W_qkv[:, 2048 + 256 * hg:2048 + 256 * hg + 256]], axis=1)
        in_maps.append({
            "x": to_bf16(x[2 * bg:2 * bg + 2]),
            "wqkv": to_bf16(wq),
            "wproj": to_bf16(W_proj[256 * hg:256 * hg + 256, :]),
            "ident": to_bf16(ident),
        })
    return in_maps


def kernel(x, W_qkv, W_proj):
    x = np.asarray(x, dtype=np.float32)
    W_qkv = np.asarray(W_qkv, dtype=np.float32)
    W_proj = np.asarray(W_proj, dtype=np.float32)
    nc = build()
    res = run_bass_kernel_spmd(nc, make_in_maps(x, W_qkv, W_proj), list(range(NC)))
    out = np.zeros((B, T, D), dtype=np.float64)
    for c in range(NC):
        bg = c // 4
        out[2 * bg:2 * bg + 2] += res.results[c]["out"].astype(np.float64)
    return out.astype(np.float32)


# revision 29
# speedup vs baseline: 1.2023x; 1.0381x over previous
"""Causal self-attention (B=4, T=2048, D=1024, H=16) on 8 TRN2 NeuronCores.

Sharding: tensor-parallel over 4 head-groups x data-parallel over 2 batch-groups.
Core c handles batches [2*(c//4), 2*(c//4)+2) and heads [4*(c%4), 4*(c%4)+4).
Each core computes a partial output projection (its 256 feature rows of W_proj);
the host sums the 4 head-group partials per batch group.

All matmul inputs are bf16 (inputs RNE-rounded on the host); accumulation is
fp32 in PSUM. bf16 streams at 1 cycle/row at EVERY free size (fp32r drops to
4 cycles/row below N=256), transposes at 1.0 vs 1.5 cycles/row, DVE/ACT get
2x 16-bit throughput, and HBM traffic halves. Softmax skips max-subtraction
(scores ~N(0,1)) so softmax(s) = exp(s)/sum(exp(s)).

Layout tricks:
- Q^T and K^T are packed two heads per 128 partitions; S^T contracts K=64
  partitions (the live head) directly, so K needs no zero-padding.
- V blocks are [64 V cols | 64 ones cols]: the PV matmul then produces the
  softmax denominator replicated across PSUM rows 64:128 for free, so the
  per-head normalization is reciprocal + multiply straight out of PSUM (no
  broadcast matmul on the tensor queue).
- Output projection for q-chunk j is emitted one j late so its PSUM/DVE work
  overlaps attention of chunk j+1.
- PSUM->SBUF evacuations are spread over scalar/vector/gpsimd so no single
  engine rate-limits the PE; x chunks are DMA-prefetched (incl. across the
  batch boundary) and the first x chunk is issued before the weights.
"""
import functools
from contextlib import ExitStack

import numpy as np
import ml_dtypes

import concourse.bacc as bacc
import concourse.tile as tile
import concourse.mybir as mybir
from concourse.bass_utils import run_bass_kernel_spmd
from concourse.masks import make_upper_triangular

F32 = mybir.dt.float32
F32R = mybir.dt.float32r
BF16 = mybir.dt.bfloat16
EXP = mybir.ActivationFunctionType.Exp

B, T, D, H, HD = 4, 2048, 1024, 16, 64
NB, NH = 2, 4            # batches / heads per core
DL = NH * HD             # local feature dim (256)
NC = 8
WCOL = 768               # per-dk weight columns: Q(256) K(256) V(256) packed


@functools.lru_cache(maxsize=1)
def build():
    nc = bacc.Bacc("TRN2", target_bir_lowering=False, debug=False, num_devices=NC)
    x_d = nc.dram_tensor("x", [NB, T, D], BF16, kind="ExternalInput").ap()
    wqkv_d = nc.dram_tensor("wqkv", [D, WCOL], BF16, kind="ExternalInput").ap()
    wproj_d = nc.dram_tensor("wproj", [DL, D], BF16, kind="ExternalInput").ap()
    ident_d = nc.dram_tensor("ident", [128, 128], BF16, kind="ExternalInput").ap()
    out_d = nc.dram_tensor("out", [NB, T, D], F32, kind="ExternalOutput").ap()

    NT5 = T // 512           # 4  (512-token super chunks)
    NTT = T // 128           # 16 (128-token chunks)
    NDK = D // 128           # 8  (feature chunks of input dim)

    with tile.TileContext(nc) as tc, ExitStack() as ctx:
        const = ctx.enter_context(tc.tile_pool(name="const", bufs=1))
        wpool = ctx.enter_context(tc.tile_pool(name="w", bufs=1))
        xin_pool = ctx.enter_context(tc.tile_pool(name="xin", bufs=4))

        ident = const.tile([128, 128], BF16)
        nc.sync.dma_start(ident[:], ident_d)

        # x chunk prefetch: one [256 tokens, D] chunk per (b, t5, half)
        xa_map = {}

        def xa_fetch(b, t5, half, eng):
            key = (b, t5, half)
            if key not in xa_map:
                xa = xin_pool.tile([128, 2 * D], BF16, tag="xa",
                                   name=f"xa{b}_{t5}_{half}")
                eng.dma_start(
                    xa[:].rearrange("p (a c) -> p a c", a=2),
                    x_d[b, 512 * t5 + 256 * half:512 * t5 + 256 * half + 256]
                    .rearrange("(a p) c -> p a c", p=128))
                xa_map[key] = xa
            return xa_map[key]

        xa_fetch(0, 0, 0, nc.scalar)
        xa_fetch(0, 0, 1, nc.scalar)

        # weights: w_sb[:, dk*WCOL + c] = wqkv[dk*128 + p, c]
        w_sb = wpool.tile([128, NDK * WCOL], BF16)
        nc.sync.dma_start(
            w_sb[:].rearrange("p (a c) -> p a c", a=NDK),
            wqkv_d.rearrange("(a p) c -> p a c", p=128))
        wp_sb = wpool.tile([128, 2 * D], BF16)
        nc.sync.dma_start(
            wp_sb[:].rearrange("p (a c) -> p a c", a=2),
            wproj_d.rearrange("(a p) c -> p a c", p=128))

        tri32 = const.tile([128, 128], F32)   # tri32[k,q] = 1.0 iff q >= k
        make_upper_triangular(nc, tri32[:], val=1.0, diag=True)
        tri = const.tile([128, 128], BF16)
        nc.vector.tensor_copy(tri[:], tri32[:])
        onesb = const.tile([128, 1024], BF16)
        nc.gpsimd.memset(onesb[:], 1.0)

        for b in range(NB):
            with tc.tile_pool(name="actv", bufs=1) as actv:
                # Q^T / K^T packed: 2 chunks of 128 rows (2 heads each)
                qt = [actv.tile([128, T], BF16, tag=f"qt{cc}", name=f"qt{cc}")
                      for cc in range(2)]
                kt = [actv.tile([128, T], BF16, tag=f"kt{cc}", name=f"kt{cc}")
                      for cc in range(2)]
                # V blocks per (token-tile ti, head h): 128 cols at (ti*4+h)*128:
                # cols 0-63 = V, cols 64-127 = ones (denominator replication)
                v_sb = actv.tile([128, NTT * NH * 128], BF16, tag="v")
                v128 = v_sb[:].rearrange("p (n c) -> p n c", c=128)

                # ---- Phase A: x^T (PE transpose), Q^T, K^T, V ----
                with tc.tile_pool(name="xt", bufs=2) as xt_pool, \
                     tc.tile_pool(name="psT", bufs=4, space="PSUM") as psT, \
                     tc.tile_pool(name="psQK", bufs=2, space="PSUM") as psQK, \
                     tc.tile_pool(name="psV", bufs=2, space="PSUM") as psV:

                    def emit_transposes(t5):
                        xas = [xa_fetch(b, t5, half, nc.scalar)
                               for half in range(2)]
                        nb_, nt5 = (b, t5 + 1) if t5 + 1 < NT5 else (b + 1, 0)
                        if nb_ < NB:
                            for half in range(2):
                                xa_fetch(nb_, nt5, half, nc.scalar)
                        xt = [xt_pool.tile([128, 512], BF16, tag=f"xt{dk}",
                                           name=f"xt{dk}") for dk in range(NDK)]
                        # x^T: 2 transposes into one PSUM tile, 1 evac each
                        ev = 0
                        for half in range(2):
                            for dk in range(NDK):
                                pt = psT.tile([128, 256], BF16, tag="pt")
                                for tt in range(2):
                                    nc.tensor.matmul(
                                        pt[:, tt * 128:tt * 128 + 128],
                                        xas[half][:, tt * D + dk * 128:tt * D + dk * 128 + 128],
                                        ident[:], is_transpose=True,
                                        start=(tt == 0), stop=(tt == 1))
                                if ev % 2 == 0:
                                    nc.scalar.copy(
                                        xt[dk][:, 256 * half:256 * half + 256], pt[:])
                                else:
                                    nc.vector.tensor_copy(
                                        xt[dk][:, 256 * half:256 * half + 256], pt[:])
                                ev += 1
                        return xt

                    def emit_qkv(t5, xt):
                        for cc in range(2):     # Q^T
                            ps = psQK.tile([128, 512], F32, tag="qk")
                            for dk in range(NDK):
                                nc.tensor.matmul(
                                    ps[:],
                                    w_sb[:, dk * WCOL + cc * 128:dk * WCOL + cc * 128 + 128],
                                    xt[dk][:],
                                    start=(dk == 0), stop=(dk == NDK - 1))
                            nc.vector.tensor_copy(
                                qt[cc][:, t5 * 512:t5 * 512 + 512], ps[:])
                        for cc in range(2):     # K^T packed (same 2-head layout)
                            ps = psQK.tile([128, 512], F32, tag="qk")
                            for dk in range(NDK):
                                nc.tensor.matmul(
                                    ps[:],
                                    w_sb[:, dk * WCOL + 256 + cc * 128:dk * WCOL + 256 + cc * 128 + 128],
                                    xt[dk][:],
                                    start=(dk == 0), stop=(dk == NDK - 1))
                            nc.vector.tensor_copy(
                                kt[cc][:, t5 * 512:t5 * 512 + 512], ps[:])
                        for tt in range(4):     # V packed (N=256)
                            ps = psV.tile([128, 256], F32, tag="v")
                            for dk in range(NDK):
                                nc.tensor.matmul(
                                    ps[:],
                                    xt[dk][:, tt * 128:tt * 128 + 128],
                                    w_sb[:, dk * WCOL + 512:dk * WCOL + 768],
                                    start=(dk == 0), stop=(dk == NDK - 1))
                            ti = t5 * 4 + tt
                            nc.vector.tensor_copy(
                                v128[:, ti * 4:ti * 4 + 4, 64:128],
                                ps[:].rearrange("p (n c) -> p n c", c=64))
                        # ones cols 0:64 of each 128-block of this chunk (the
                        # denominator must land at PSUM partition offset 0:
                        # custom-DVE reciprocal drops nonzero partition offsets)
                        nc.vector.tensor_copy(
                            v128[:, 16 * t5:16 * (t5 + 1), 0:64],
                            onesb[:].rearrange("p (n c) -> p n c", c=64))

                    for t5 in range(NT5):
                        emit_qkv(t5, emit_transposes(t5))

                # ---- Phase B: attention + (pipelined) output projection ----
                # All S^T matmuls of a (h,j) first (exps stream on ACT), then
                # all PV matmuls: long same-shape runs keep the PE at full clock.
                with tc.tile_pool(name="psS", bufs=2, space="PSUM") as psS_pool, \
                     tc.tile_pool(name="psY", bufs=2, space="PSUM") as psY_pool, \
                     tc.tile_pool(name="psO", bufs=2, space="PSUM") as psO_pool, \
                     tc.tile_pool(name="pP", bufs=9) as pP, \
                     tc.tile_pool(name="ytp", bufs=2) as ytp, \
                     tc.tile_pool(name="ost", bufs=2) as ost_pool, \
                     tc.tile_pool(name="rbp", bufs=3) as rbp:

                    def emit_proj(j, yt):
                        ev = 0
                        for g2 in range(2):
                            ostage = ost_pool.tile([128, 2 * D], F32, tag="o")
                            for a in range(2):
                                tt = 2 * g2 + a
                                for nn2 in range(2):
                                    ps = psO_pool.tile([128, 512], F32, tag="o")
                                    for ff in range(2):
                                        nc.tensor.matmul(
                                            ps[:],
                                            yt[ff][:, 128 * tt:128 * tt + 128],
                                            wp_sb[:, ff * D + 512 * nn2:ff * D + 512 * nn2 + 512],
                                            start=(ff == 0), stop=(ff == 1))
                                    dst = ostage[:, a * D + 512 * nn2:a * D + 512 * nn2 + 512]
                                    nc.vector.tensor_copy(dst, ps[:])
                                    ev += 1
                            nc.sync.dma_start(
                                out_d[b, 512 * j + 256 * g2:512 * j + 256 * g2 + 256]
                                .rearrange("(a p) c -> p a c", p=128),
                                ostage[:].rearrange("p (a c) -> p a c", a=2))

                    # prefetch next batch's first x chunk during attention
                    if b + 1 < NB:
                        for half in range(2):
                            xa_fetch(b + 1, 0, half, nc.gpsimd)

                    prev = None
                    for j in range(NT5):
                        yt = [ytp.tile([128, 512], BF16, tag=f"yt{ff}",
                                       name=f"yt{ff}") for ff in range(2)]
                        for h in range(NH):
                            ro = 64 * (h % 2)
                            cc = h // 2
                            nk = 4 * j + 4
                            offs = [128 * (i - 4 * j) if i - 4 * j > 0 else 0
                                    for i in range(nk)]
                            Ps = []
                            for m in range(nk // 2):
                                psS = psS_pool.tile([128, 1024], F32, tag="s",
                                                    name=f"psS{m}")
                                P = pP.tile([128, 1024], BF16, tag="p",
                                            name=f"P{m}")
                                Ps.append(P)
                                for c in (0, 1):
                                    i = 2 * m + c
                                    off = offs[i]
                                    nc.tensor.matmul(
                                        psS[:, c * 512 + off:(c + 1) * 512],
                                        kt[cc][ro:ro + 64, 128 * i:128 * i + 128],
                                        qt[cc][ro:ro + 64, 512 * j + off:512 * (j + 1)],
                                        start=True, stop=True)
                                if 2 * m + 1 < 4 * j or 2 * m == 4 * j:
                                    # t0/t1 diagonal pair: exp the whole tile in
                                    # one op; cols 512..640 are never read by PV
                                    nc.scalar.activation(P[:], psS[:], EXP, scale=0.125)
                                else:
                                    for c in (0, 1):
                                        off = offs[2 * m + c]
                                        nc.scalar.activation(
                                            P[:, c * 512 + off:(c + 1) * 512],
                                            psS[:, c * 512 + off:(c + 1) * 512],
                                            EXP, scale=0.125)
                                for c in (0, 1):
                                    i = 2 * m + c
                                    if i >= 4 * j:
                                        off = offs[i]
                                        nc.vector.tensor_mul(
                                            P[:, c * 512 + off:c * 512 + off + 128],
                                            P[:, c * 512 + off:c * 512 + off + 128],
                                            tri[:])
                            psY = psY_pool.tile([128, 512], F32, tag="y")
                            for i in range(nk):
                                off = offs[i]
                                nc.tensor.matmul(
                                    psY[:, off:512],
                                    v_sb[:, 512 * i + 128 * h:512 * i + 128 * h + 128],
                                    Ps[i // 2][:, (i % 2) * 512 + off:(i % 2 + 1) * 512],
                                    start=(i == 0), stop=(i == nk - 1))
                            # normalize: denominator sits replicated in rows
                            # 0:64 of psY (ones cols of V) — no broadcast.
                            rb = rbp.tile([64, 512], F32, tag="rb")
                            nc.vector.reciprocal_approx_fast(rb[:], psY[0:64, :])
                            nc.vector.tensor_mul(
                                yt[h // 2][ro:ro + 64, :],
                                psY[64:128, :], rb[:])
                        if prev is not None:
                            emit_proj(prev[0], prev[1])
                        prev = (j, yt)
                    emit_proj(prev[0], prev[1])

    nc.compile()
    return nc


def to_bf16(a: np.ndarray) -> np.ndarray:
    return np.ascontiguousarray(a).astype(ml_dtypes.bfloat16)


def make_in_maps(x, W_qkv, W_proj):
    ident = np.eye(128, dtype=np.float32)
    in_maps = []
    for c in range(NC):
        bg, hg = c // 4, c % 4
        wq = np.concatenate(
            [W_qkv[:, 256 * hg:256 * hg + 256],
             W_qkv[:, 1024 + 256 * hg:1024 + 256 * hg + 256],
             W_qkv[:, 2048 + 256 * hg:2048 + 256 * hg + 256]], axis=1)
        in_maps.append({
            "x": to_bf16(x[2 * bg:2 * bg + 2]),
            "wqkv": to_bf16(wq),
            "wproj": to_bf16(W_proj[256 * hg:256 * hg + 256, :]),
            "ident": to_bf16(ident),
        })
    return in_maps


def kernel(x, W_qkv, W_proj):
    x = np.asarray(x, dtype=np.float32)
    W_qkv = np.asarray(W_qkv, dtype=np.float32)
    W_proj = np.asarray(W_proj, dtype=np.float32)
    nc = build()
    res = run_bass_kernel_spmd(nc, make_in_maps(x, W_qkv, W_proj), list(range(NC)))
    out = np.zeros((B, T, D), dtype=np.float64)
    for c in range(NC):
        bg = c // 4
        out[2 * bg:2 * bg + 2] += res.results[c]["out"].astype(np.float64)
    return out.astype(np.float32)
